# revision 1
# baseline (speedup 1.0000x reference)
"""Distance transform kernel for Trainium2 (8 NeuronCores, SPMD).

Computes, for each pixel (i,j) of a 128x128 grid, the min Euclidean distance
to any "boundary" pixel (feature_map > 0.5, pooled over batch/channel), and
broadcasts the result over the batch dimension.

Instead of the naive [H,W,H,W] pairwise min (268M candidate distances), uses
the exact separable two-phase Euclidean distance transform:
  phase 1: per-row 1D distance d1[h,j] = min_w |j-w| over boundary pixels of
           row h -- two hardware scans (state = min(state+1, pen[t])),
           forward and (via a reversed access pattern) backward.
  phase 2: dist^2[i,j] = min_h ( (i-h)^2 + d1[h,j]^2 ) -- min over h,
           exact for integer grids.

Sharding (halo): core c computes output rows i in [16c, 16c+16) and only
receives the HR-row neighborhood true-h in [16c-WIN/2, 16c-WIN/2+HR) of
the feature map (zero-padded outside the grid; zero rows have no boundary
pixels and yield sentinel distances that never win the min). In local
coordinates h' = h - (16c-WIN/2), every core runs the identical program
with the phase-2 window h' in [il, il+WIN) for local output row il -- this
window covers |h - i| <= WIN/2 - 1 = DMAX, so the result is exact whenever
the true distance field is everywhere <= DMAX (d(i,j) >= |i - h*| makes
max(dist) <= DMAX a sufficient host-side check). On failure the caller
falls back to a full-width program, keeping the kernel correct for any
input. For this problem's inputs (mask density ~255/256) distances are
~1-2, so the fast path always applies.

Output is batch-replicated, so no collectives are needed; the host gathers
the per-core [128,16] column blocks, transposes, and broadcasts over batch.
"""

import ml_dtypes
import numpy as np

import concourse.bacc as bacc
import concourse.masks as masks
import concourse.mybir as mybir
import concourse.tile as tile
from concourse.bass_utils import run_bass_kernel_spmd

H = 128          # grid height == width
B = 8            # batch
NCORES = 8
TI = H // NCORES  # output rows per core
HR = 24          # halo rows per core (windowed program)
WIN = 8          # phase-2 h-window per output row
DMAX = 3.0       # windowed result exact iff max distance <= DMAX

DT = mybir.dt.float32
SENTINEL = 1.0e4   # penalty for non-boundary pixels (>> max real distance)
SCAN_INIT = 1.0e9  # initial scan state
RED_INIT = 1.0e30  # pad value for log-step min fallback

import os as _os
USE_SCAN = _os.environ.get("K_USE_SCAN", "1") == "1"   # tensor_tensor_scan
USE_RSCAN = _os.environ.get("K_USE_RSCAN", "1") == "1"  # reversed-AP scan

_CACHE: dict = {}


def _logstep_prefix_min(nc, pool, src, rows, sign, tag):
    """Suffix (sign=+1) / prefix (sign=-1) min along the free dim via
    log-step shifted mins on a padded ping-pong buffer."""
    Alu = mybir.AluOpType
    pad = 64
    a = pool.tile([rows, H + pad], DT, tag=f"lsa{tag}")
    b = pool.tile([rows, H + pad], DT, tag=f"lsb{tag}")
    if sign > 0:
        data, padsl = slice(0, H), slice(H, H + pad)
    else:
        data, padsl = slice(pad, H + pad), slice(0, pad)
    nc.gpsimd.memset(a[:, padsl], RED_INIT)
    nc.gpsimd.memset(b[:, padsl], RED_INIT)
    nc.vector.tensor_copy(a[:, data], src)
    s = 1
    off = pad if sign < 0 else 0
    while s < H:
        sh = slice(off + sign * s, off + sign * s + H)
        nc.vector.tensor_tensor(b[:, data], a[:, data], a[:, sh], op=Alu.min)
        a, b = b, a
        s *= 2
    return a[:, data]


def _dmas(nc, pool, windowed, rows, fm_d, ib_d):
    """Issue the input DMAs. Emitted before the on-device constants so the
    Pool SWDGE descriptor generation isn't queued behind them."""
    hb = B // 2
    fdt = fm_d.dtype
    fm3 = fm_d.rearrange("b c h w -> h (b c) w")  # [rows, B, H]
    fmb = pool.tile([rows, hb, H], fdt, tag="fmb")
    nc.gpsimd.dma_start(fmb[:], fm3[:, hb:B])
    fma = pool.tile([rows, hb, H], fdt, tag="fma")
    nc.sync.dma_start(fma[:], fm3[:, 0:hb])
    ibx = None
    if not windowed:
        # ibias via the second HWDGE queue (ACT)
        ibx = pool.tile([H, 2 * TI], DT, tag="ibx")
        nc.scalar.dma_start(ibx[:], ib_d)
    return fma, fmb, ibx


def _body(nc, tc, pool, psumpool, windowed, rows, win, fm_d, ib_d, out_d,
          ident, iota_f, iotasq, psq, ones, sent, dmas=None):
    Alu = mybir.AluOpType
    if dmas is None:
        dmas = _dmas(nc, pool, windowed, rows, fm_d, ib_d)
    fma, fmb, ibx = dmas
    if not windowed:
        m2i = ibx[:, 0:TI]
        isq = ibx[:, TI:2 * TI]

    # union over batch: wide max tree, halves overlap the DMAs
    fdt = fma[:].dtype
    ma = pool.tile([rows, 2 * H], fdt, tag="ma")
    fma2 = fma[:].rearrange("p b w -> p (b w)")
    fmb2 = fmb[:].rearrange("p b w -> p (b w)")
    nc.vector.tensor_tensor(ma[:], fma2[:, 0:2 * H],
                            fma2[:, 2 * H:4 * H], op=Alu.max)
    mb = pool.tile([rows, 2 * H], fdt, tag="mb")
    nc.vector.tensor_tensor(mb[:], fmb2[:, 0:2 * H],
                            fmb2[:, 2 * H:4 * H], op=Alu.max)
    m2t = pool.tile([rows, 2 * H], fdt, tag="m2t")
    nc.vector.tensor_tensor(m2t[:], ma[:], mb[:], op=Alu.max)
    mx = pool.tile([rows, H], fdt, tag="mx")
    nc.vector.tensor_tensor(mx[:], m2t[:, 0:H], m2t[:, H:2 * H], op=Alu.max)

    # penalty: 0 where boundary, SENTINEL elsewhere. bf16 path: boundary
    # is mx >= 0.5 (truncated input); f32 path: boundary is mx > 0.5.
    pdt = fdt if windowed else DT
    pen = pool.tile([rows, H], pdt, tag="pen")
    nc.vector.tensor_scalar(out=pen[:], in0=mx[:], scalar1=0.5,
                            scalar2=sent[0:rows, 0:1],
                            op0=(Alu.is_lt if windowed else Alu.is_le),
                            op1=Alu.mult)

    # phase 1: 1D distance per row via hardware scans (state is fp32
    # internally; bf16 outputs are exact for integer distances <= 256)
    fsc = pool.tile([rows, H], pdt, tag="fsc")
    d1 = pool.tile([rows, H], pdt, tag="d1")
    if USE_SCAN:
        nc.vector.tensor_tensor_scan(fsc[:], ones[0:rows, :], pen[:],
                                     SCAN_INIT, op0=Alu.add, op1=Alu.min)
        if USE_RSCAN:
            bsc = pool.tile([rows, H], pdt, tag="bscr")
            nc.vector.tensor_tensor_scan(bsc[:], ones[0:rows, :],
                                         pen[:, ::-1], SCAN_INIT,
                                         op0=Alu.add, op1=Alu.min)
            nc.vector.tensor_tensor(d1[:], fsc[:], bsc[:, ::-1], op=Alu.min)
        else:
            v = pool.tile([rows, H], DT, tag="v")
            nc.vector.tensor_tensor(v[:], pen[:], iota_f[0:rows, :],
                                    op=Alu.add)
            vsf = _logstep_prefix_min(nc, pool, v[:], rows, +1, "s")
            bsc = pool.tile([rows, H], DT, tag="bsc")
            nc.vector.tensor_tensor(bsc[:], vsf, iota_f[0:rows, :],
                                    op=Alu.subtract)
            nc.vector.tensor_tensor(d1[:], fsc[:], bsc[:], op=Alu.min)
    else:
        u = pool.tile([rows, H], DT, tag="u")
        nc.vector.tensor_tensor(u[:], pen[:], iota_f[0:rows, :],
                                op=Alu.subtract)
        upf = _logstep_prefix_min(nc, pool, u[:], rows, -1, "p")
        nc.vector.tensor_tensor(fsc[:], upf, iota_f[0:rows, :], op=Alu.add)
        v = pool.tile([rows, H], DT, tag="v")
        nc.vector.tensor_tensor(v[:], pen[:], iota_f[0:rows, :], op=Alu.add)
        vsf = _logstep_prefix_min(nc, pool, v[:], rows, +1, "s")
        bsc = pool.tile([rows, H], DT, tag="bsc")
        nc.vector.tensor_tensor(bsc[:], vsf, iota_f[0:rows, :],
                                op=Alu.subtract)
        nc.vector.tensor_tensor(d1[:], fsc[:], bsc[:], op=Alu.min)

    # transpose d1 (PE, pass-through so PSUM dtype matches d1), square
    # it (ACT, PSUM->SBUF, converts to f32 -- exact for ints <= 256)
    pt = psumpool.tile([H, rows], pdt, tag="pt")
    nc.tensor.transpose(pt[:], d1[:], ident[:])
    t2 = pool.tile([H, rows], DT, tag="t2")  # d1[h,j]^2 at [j,h]
    nc.scalar.square(t2[:], pt[:])

    nd = 10                       # phase-2 output rows on DVE
    np_ = TI - nd                 # phase-2 output rows on Pool
    bigt = pool.tile([H, TI * win], DT, tag="bigt")
    biga = bigt[:, 0:nd * win]
    bigb = bigt[:, nd * win:TI * win]
    d2 = pool.tile([H, TI], DT, tag="d2")

    if windowed:
        # phase 2: cand[j, il, k] = d1T^2[j, il+k] + (k - WIN/2)^2; the
        # parabola row is il-independent in local coordinates, so ALL
        # output rows of an engine are one wide add over an overlapping
        # strided view of t2 (block step 1, inner step 1).
        import bass_rust
        t2ap = t2[:]

        def t2_blocks(first, count):
            return bass_rust.AP(
                t2ap.tensor, t2ap.offset + first,
                [list(t2ap.ap[0]), [1, count], [1, win]])

        nc.vector.tensor_tensor(
            biga.rearrange("p (a k) -> p a k", k=win),
            t2_blocks(0, nd),
            psq[:, 0:nd * win].rearrange("p (a k) -> p a k", k=win),
            op=Alu.add)
        nc.gpsimd.tensor_tensor(
            bigb.rearrange("p (a k) -> p a k", k=win),
            t2_blocks(nd, np_),
            psq[:, 0:np_ * win].rearrange("p (a k) -> p a k", k=win),
            op=Alu.add)
    else:
        # phase 2 via i-dependent scalars:
        # cand = (iota * -2i) + (d1T^2 + h^2); +i^2 added at the end
        t2h = pool.tile([H, rows], DT, tag="t2h")
        nc.vector.tensor_tensor(t2h[:], t2[:], iotasq[:, 0:rows], op=Alu.add)
        for il in range(nd):
            nc.vector.scalar_tensor_tensor(
                out=biga[:, il * win:(il + 1) * win], in0=iota_f[:, 0:win],
                scalar=m2i[:, il:il + 1], in1=t2h[:, 0:win],
                op0=Alu.mult, op1=Alu.add)
        for il in range(nd, TI):
            k = il - nd
            sl = slice(k * win, (k + 1) * win)
            nc.gpsimd.tensor_scalar(
                out=bigb[:, sl], in0=iota_f[:, 0:win],
                scalar1=m2i[:, il:il + 1], scalar2=None, op0=Alu.mult)
            nc.gpsimd.tensor_tensor(bigb[:, sl], bigb[:, sl],
                                    t2h[:, 0:win], op=Alu.add)

    nc.vector.tensor_reduce(
        d2[:, 0:nd], biga.rearrange("p (i h) -> p i h", h=win),
        axis=mybir.AxisListType.X, op=Alu.min)
    nc.vector.tensor_reduce(
        d2[:, nd:TI], bigb.rearrange("p (i h) -> p i h", h=win),
        axis=mybir.AxisListType.X, op=Alu.min)

    if not windowed:
        d2f = pool.tile([H, TI], DT, tag="d2f")
        nc.vector.tensor_tensor(d2f[:], d2[:], isq[:], op=Alu.add)
        d2 = d2f
    res = pool.tile([H, TI], DT, tag="res")
    nc.scalar.sqrt(res[:], d2[:])
    nc.sync.dma_start(out_d, res[:])


def _build_program(windowed: bool, repeat: int = 1, hw_loop_iters: int = 0):
    """One SPMD program. windowed=True: fm input is the per-core halo
    [B,1,HR,H] and phase 2 uses WIN-wide h-windows. windowed=False: fm is
    the full [B,1,H,H] image and phase 2 scans all 128 rows. repeat>1
    re-runs the whole body (incl. DMAs) for marginal-time measurement."""
    Alu = mybir.AluOpType
    rows = HR if windowed else H          # mask rows processed on this core
    win = WIN if windowed else H          # phase-2 candidate rows per output
    # windowed path ships the feature map as truncated bf16: the input is
    # only ever compared against 0.5 and trunc16(v) >= 0.5 <=> v > 0.5
    # (v == 0.5 exactly is host-guarded); bf16 gets the DVE 2x mode on the
    # max tree, the widest ops on the critical path.
    fdt = mybir.dt.bfloat16 if windowed else DT
    nc = bacc.Bacc("TRN2", target_bir_lowering=False, debug=False,
                   num_devices=NCORES)
    fm_d = nc.dram_tensor("fm", [B, 1, rows, H], fdt,
                          kind="ExternalInput").ap()
    ib_d = None
    if not windowed:
        # per-core side input: columns [0:TI] = -2*i, [TI:2TI] = i^2
        ib_d = nc.dram_tensor("ibias", [H, 2 * TI], DT,
                              kind="ExternalInput").ap()
    out_d = nc.dram_tensor("out", [H, TI], DT, kind="ExternalOutput").ap()

    with tile.TileContext(nc) as tc:
        with tc.tile_pool(name="main", bufs=1) as pool, \
             tc.tile_pool(name="psum", bufs=1, space="PSUM") as psumpool:

            dmas = None
            if not hw_loop_iters and repeat == 1:
                dmas = _dmas(nc, pool, windowed, rows, fm_d, ib_d)

            # constants built on device (during the first DMAs)
            cdt = mybir.dt.bfloat16 if windowed else DT
            ident = pool.tile([rows, rows], cdt, tag="ident")
            masks.make_identity(nc, ident[:])
            # sentinel via an early live Sqrt: makes the ACT func-table
            # pass load the sqrt set (which also contains Square) once,
            # instead of a mid-kernel 1.3us table switch before the final
            # sqrt. pen consumes it as a per-partition scalar.
            sent2 = pool.tile([H, 1], DT, tag="sent2")
            nc.gpsimd.memset(sent2[:], SENTINEL * SENTINEL)
            sent = pool.tile([H, 1], DT, tag="sent")
            nc.scalar.sqrt(sent[:], sent2[:])
            iota_f = iotasq = None
            if not (windowed and USE_SCAN and USE_RSCAN):
                iota_i = pool.tile([H, H], mybir.dt.int32, tag="iota_i")
                nc.gpsimd.iota(iota_i[:], pattern=[[1, H]], base=0,
                               channel_multiplier=0)
                iota_f = pool.tile([H, H], DT, tag="iota_f")
                nc.vector.tensor_copy(iota_f[:], iota_i[:])
                iotasq = pool.tile([H, H], DT, tag="iotasq")
                nc.scalar.square(iotasq[:], iota_f[:])
            if windowed:
                # psq[:, a*WIN + k] = (k - WIN/2)^2 for every block a: the
                # (i-h)^2 parabola is the same WIN-vector for every output
                # row in local coordinates, replicated TI times so phase 2
                # can consume it in one wide op per engine.
                psq_i = pool.tile([H, TI * WIN], mybir.dt.int32, tag="psq_i")
                nc.gpsimd.iota(psq_i[:], pattern=[[0, TI], [1, WIN]],
                               base=-WIN // 2, channel_multiplier=0)
                psq_f = pool.tile([H, TI * WIN], DT, tag="psq_f")
                nc.vector.tensor_copy(psq_f[:], psq_i[:])
                psq = pool.tile([H, TI * WIN], DT, tag="psq")
                nc.scalar.square(psq[:], psq_f[:])
            ones = pool.tile([rows, H], cdt, tag="ones")
            nc.gpsimd.memset(ones[:], 1.0)

            if hw_loop_iters:
                with tc.For_i(0, hw_loop_iters, 1):
                    _body(nc, tc, pool, psumpool, windowed, rows, win,
                          fm_d, ib_d, out_d, ident, iota_f, iotasq,
                          psq if windowed else None, ones, sent)
            else:
                for _rep in range(repeat):
                    _body(nc, tc, pool, psumpool, windowed, rows, win,
                          fm_d, ib_d, out_d, ident, iota_f, iotasq,
                          psq if windowed else None, ones, sent,
                          dmas=dmas if _rep == 0 else None)

    nc.compile()
    return nc


def _get_program(windowed: bool):
    key = "win" if windowed else "full"
    if key not in _CACHE:
        _CACHE[key] = _build_program(windowed)
    return _CACHE[key]


def _in_maps(feature_map: np.ndarray, windowed: bool):
    maps = []
    for c in range(NCORES):
        if windowed:
            # halo rows are true h in [16c-WIN/2, ...), zero-padded outside
            # the grid (zero rows have no boundary pixels). Shipped as
            # truncated bf16: v > 0.5 <=> trunc16(v) >= 0.5 for v != 0.5.
            lo = 16 * c - WIN // 2
            fm_c = np.zeros((B, 1, HR, H), np.float32)
            s, e = max(0, lo), min(H, lo + HR)
            fm_c[:, :, s - lo:e - lo, :] = feature_map[:, :, s:e, :]
            fm_bf = (np.ascontiguousarray(fm_c).view(np.uint32) >> 16) \
                .astype(np.uint16).view(ml_dtypes.bfloat16)
            maps.append({"fm": fm_bf})
        else:
            iv = np.arange(c * TI, (c + 1) * TI, dtype=np.float32)
            row = np.concatenate([-2.0 * iv, iv * iv])
            maps.append({
                "fm": np.ascontiguousarray(feature_map),
                "ibias": np.ascontiguousarray(
                    np.broadcast_to(row[None, :], (H, 2 * TI))),
            })
    return maps


def _run(feature_map, windowed, trace=False):
    nc = _get_program(windowed)
    out = run_bass_kernel_spmd(nc, _in_maps(feature_map, windowed),
                               list(range(NCORES)), trace=trace)
    _CACHE["last_result"] = out
    # per-core block c is [128(j), 16(i_local)] with i = 16c + i_local
    cols = np.concatenate([r["out"] for r in out.results], axis=1)
    return cols.T  # [i, j]


def kernel(feature_map: np.ndarray, _trace: bool = False):
    fm = np.ascontiguousarray(np.asarray(feature_map, dtype=np.float32))
    assert fm.shape == (B, 1, H, H), fm.shape
    if np.any(fm == np.float32(0.5)):
        # bf16-truncation trick needs v != 0.5 exactly; exact full program
        dist = _run(fm, windowed=False, trace=_trace)
        return np.ascontiguousarray(
            np.broadcast_to(dist[None, None], (B, 1, H, H))
            .astype(np.float32))
    dist = _run(fm, windowed=True, trace=_trace)
    if not np.all(dist <= DMAX + 0.01):  # margin for ACT sqrt rounding
        # windowed result not provably exact -> exact full-width program
        dist = _run(fm, windowed=False, trace=_trace)
    return np.ascontiguousarray(
        np.broadcast_to(dist[None, None], (B, 1, H, H)).astype(np.float32))



# revision 10
# speedup vs baseline: 1.0588x; 1.0588x over previous
"""Distance transform kernel for Trainium2 (8 NeuronCores, SPMD).

Computes, for each pixel (i,j) of a 128x128 grid, the min Euclidean distance
to any "boundary" pixel (feature_map > 0.5, pooled over batch/channel), and
broadcasts the result over the batch dimension.

Three device programs, fastest first, each exact on a host-checkable domain:

1. "cross" (primary): binary distance transform. dist(i,j) = 0 if (i,j) is
   boundary, 1 if any 4-neighbor is boundary, else a sentinel. Exact whenever
   the true distance field is everywhere <= 1 (then dist == dist^2, so no
   sqrt is needed and the device's {0,1} output IS the final answer). The
   host validates all outputs are in {0,1}; any sentinel (true dist > 1)
   falls back to program 2. Six DVE ops total, no PE/ACT usage:
     mx  = max over batch (one innermost-axis reduce; host packs the input
           as [h, w, b] so batch is innermost)
     pen = (mx < 0.5) * SENT           (0 on boundary, SENT elsewhere)
     u   = min(pen[.,j-1], pen[.,j+1])  (free-dim shifted views, SENT-padded)
     vv  = min(pen[h-1,.], pen[h+1,.])  (partition-shifted views of the halo)
     nb  = min(u, vv)                   (best 4-neighbor)
     out = min(nb + 1, pen)             ({0, 1, ~SENT})
   Sharding: core c owns output rows [16c, 16c+16) and receives an 18-row
   halo [16c-1, 16c+17) (zero-padded outside the grid; zero rows are
   non-boundary and never win).

2. "windowed": exact separable two-phase Euclidean DT with an 8-wide phase-2
   window; exact iff max distance <= 3 (host-checked). See _build_program.

3. "full": full-width phase 2, exact for any input.

Inputs ship as truncated bf16: v > 0.5  <=>  trunc16(v) >= 0.5 for v != 0.5
(v == 0.5 exactly is host-guarded straight to program 3).

Output is batch-replicated, so no collectives are needed.
"""

import ml_dtypes
import numpy as np

import concourse.bacc as bacc
import concourse.masks as masks
import concourse.mybir as mybir
import concourse.tile as tile
from concourse.bass_utils import run_bass_kernel_spmd

H = 128          # grid height == width
B = 8            # batch
NCORES = 8
TI = H // NCORES  # output rows per core
HR = 24          # halo rows per core (windowed program)
WIN = 8          # phase-2 h-window per output row
DMAX = 3.0       # windowed result exact iff max distance <= DMAX
CR = TI + 2      # halo rows per core (cross program)

DT = mybir.dt.float32
BF = mybir.dt.bfloat16
SENTINEL = 1.0e4   # penalty for non-boundary pixels (>> max real distance)
CSENT = 16384.0    # cross-program sentinel (exact power of two in bf16)
SCAN_INIT = 1.0e9  # initial scan state
RED_INIT = 1.0e30  # pad value for log-step min fallback

import os as _os
USE_SCAN = _os.environ.get("K_USE_SCAN", "1") == "1"   # tensor_tensor_scan
USE_RSCAN = _os.environ.get("K_USE_RSCAN", "1") == "1"  # reversed-AP scan

_CACHE: dict = {}


# --------------------------------------------------------------------------
# Program 1: cross (binary DT, exact iff max dist <= 1)
# --------------------------------------------------------------------------

XC = CR + 2 * TI  # free-dim columns per (w, b): 18 center + 16 left + 16 right


def _cross_body(nc, pool, fm_d, out_d, res, ctx0, sem_out):
    Alu = mybir.AluOpType
    fmt = pool.tile([128, XC * B], BF, tag="xfmt")
    nc.sync.dma_start(fmt[:], fm_d)

    # union over batch: one reduce over the innermost (batch) axis.
    # Partition = w (all 128 lanes); free cols = 18 center halo rows,
    # then 16 left-shifted (w-1) and 16 right-shifted (w+1) output rows.
    mx = pool.tile([128, XC], BF, tag="xmx")
    nc.vector.tensor_reduce(
        mx[:], fmt[:].rearrange("p (x b) -> p x b", b=B),
        axis=mybir.AxisListType.X, op=Alu.max)

    # penalty: 0 on boundary (mx >= 0.5 on truncated bf16), SENT elsewhere
    pc = pool.tile([128, CR], BF, tag="xpc")
    nc.vector.tensor_scalar(out=pc[:], in0=mx[:, 0:CR],
                            scalar1=0.5, scalar2=CSENT,
                            op0=Alu.is_lt, op1=Alu.mult)
    plr = pool.tile([128, 2 * TI], BF, tag="xplr")
    nc.vector.tensor_scalar(out=plr[:], in0=mx[:, CR:XC],
                            scalar1=0.5, scalar2=CSENT,
                            op0=Alu.is_lt, op1=Alu.mult)

    # best 4-neighbor: vertical from shifted center views, horizontal from
    # the host-shifted copies
    vu = pool.tile([128, TI], BF, tag="xvu")
    nc.vector.tensor_tensor(vu[:], pc[:, 0:TI], pc[:, 2:CR], op=Alu.min)
    hm = pool.tile([128, TI], BF, tag="xhm")
    nc.vector.tensor_tensor(hm[:], plr[:, 0:TI], plr[:, TI:2 * TI],
                            op=Alu.min)
    nb = pool.tile([128, TI], BF, tag="xnb")
    nc.vector.tensor_tensor(nb[:], vu[:], hm[:], op=Alu.min)
    nc.vector.scalar_tensor_tensor(out=res[:], in0=nb[:], scalar=1.0,
                                   in1=pc[:, 1:TI + 1],
                                   op0=Alu.add, op1=Alu.min)
    # output-writeback descriptors, pre-generated on Pool while the input
    # DMA / compute run (the res read defers to the trigger, so the prep
    # itself only syncs on ctx0). Emitted after the res op so the RAW edge
    # exists and lands on the trigger.
    nc.gpsimd.kv_writeback(
        out_d, res[:].rearrange("p (a b n) -> p a b n", a=1, b=1),
        ctx0[:], prepare_only=True, sem=sem_out, queue_num=0)
    # fire the output writeback (waits on res via the deferred data dep);
    # skips the HWDGE queue and the DGE start delay entirely
    nc.gpsimd.trigger_dma(count=None, queue_num=0)


def _build_cross(hw_loop_iters: int = 0):
    """Cross program, partition dim = w. fm input is [W, XC, B] bf16 per
    core: for each column w, the 18-row center halo plus the 16 output
    rows of columns w-1 and w+1 (host-shifted copies; zeros outside the
    grid), batch innermost. Output is [W, TI] f32 (transposed: host takes
    .T): values in {0, 1} when the result is valid, >= CSENT - 1 wherever
    the true distance exceeds 1. The output lands via a SWDGE kv_writeback
    (plain indexed write) whose descriptors are pre-generated on the
    otherwise-idle Pool engine while the input DMA is in flight, so the
    post-compute tail is just trigger -> transfer -> completion sem."""
    from concourse.tile_scheduler import PROC_NAME_TO_IDX
    nc = bacc.Bacc("TRN2", target_bir_lowering=False, debug=False,
                   num_devices=NCORES)
    fm_d = nc.dram_tensor("fm", [128, XC * B], BF, kind="ExternalInput").ap()
    out_d = nc.dram_tensor("out", [1, 128, 1, TI], DT,
                           kind="ExternalOutput").ap()

    with tile.TileContext(nc) as tc:
        with tc.tile_pool(name="main", bufs=1) as pool:
            assert tc.sems is not None
            # the DMA-completion sem baked into the writeback descriptors
            # must be the DMASW lane sem that tile's pass 1 assigns
            # (consumers wait on it); this program has exactly one Pool DMA
            # inst, so the round-robin deterministically lands on lane 0
            sem_out = tc.sems[PROC_NAME_TO_IDX["DMASW0"]]
            res = pool.tile([128, TI], DT, tag="xres")
            ctx0 = pool.tile([128, 1], mybir.dt.int32, tag="xctx")
            nc.gpsimd.memset(ctx0[:], 0)

            if hw_loop_iters:
                with tc.For_i(0, hw_loop_iters, 1):
                    _cross_body(nc, pool, fm_d, out_d, res, ctx0, sem_out)
            else:
                _cross_body(nc, pool, fm_d, out_d, res, ctx0, sem_out)

    nc.compile()
    return nc


def _cross_in_maps(feature_map: np.ndarray):
    # [B,1,H,W] f32 -> per-core [W, XC, B] truncated bf16: center halo rows
    # 16c-1 .. 16c+17, then the owned rows of columns w-1 and w+1.
    fm_bf = (np.ascontiguousarray(feature_map[:, 0]).view(np.uint32) >> 16) \
        .astype(np.uint16).view(ml_dtypes.bfloat16)  # [B, H, W]
    maps = []
    for c in range(NCORES):
        lo = TI * c - 1
        a = np.zeros((B, XC, H), ml_dtypes.bfloat16)  # [b, x, w]
        s, e = max(0, lo), min(H, lo + CR)
        a[:, s - lo:e - lo, :] = fm_bf[:, s:e, :]
        own = fm_bf[:, TI * c:TI * (c + 1), :]        # [B, TI, W]
        a[:, CR:CR + TI, 1:] = own[:, :, :-1]         # left neighbor w-1
        a[:, CR + TI:XC, :-1] = own[:, :, 1:]         # right neighbor w+1
        # -> [w, x, b] flat
        maps.append({"fm": np.ascontiguousarray(
            a.transpose(2, 1, 0)).reshape(128, XC * B)})
    return maps


def _run_cross(feature_map, trace=False):
    if "cross" not in _CACHE:
        _CACHE["cross"] = _build_cross()
    nc = _CACHE["cross"]
    out = run_bass_kernel_spmd(nc, _cross_in_maps(feature_map),
                               list(range(NCORES)), trace=trace)
    _CACHE["last_result"] = out
    # per-core block c is [w, i_local] with i = 16c + i_local
    return np.concatenate(
        [r["out"].reshape(128, TI).T for r in out.results], axis=0)


# --------------------------------------------------------------------------
# Programs 2 & 3: windowed / full separable two-phase Euclidean DT
# --------------------------------------------------------------------------

def _logstep_prefix_min(nc, pool, src, rows, sign, tag):
    """Suffix (sign=+1) / prefix (sign=-1) min along the free dim via
    log-step shifted mins on a padded ping-pong buffer."""
    Alu = mybir.AluOpType
    pad = 64
    a = pool.tile([rows, H + pad], DT, tag=f"lsa{tag}")
    b = pool.tile([rows, H + pad], DT, tag=f"lsb{tag}")
    if sign > 0:
        data, padsl = slice(0, H), slice(H, H + pad)
    else:
        data, padsl = slice(pad, H + pad), slice(0, pad)
    nc.gpsimd.memset(a[:, padsl], RED_INIT)
    nc.gpsimd.memset(b[:, padsl], RED_INIT)
    nc.vector.tensor_copy(a[:, data], src)
    s = 1
    off = pad if sign < 0 else 0
    while s < H:
        sh = slice(off + sign * s, off + sign * s + H)
        nc.vector.tensor_tensor(b[:, data], a[:, data], a[:, sh], op=Alu.min)
        a, b = b, a
        s *= 2
    return a[:, data]


def _dmas(nc, pool, windowed, rows, fm_d, ib_d):
    """Issue the input DMAs. Emitted before the on-device constants so the
    Pool SWDGE descriptor generation isn't queued behind them."""
    hb = B // 2
    fdt = fm_d.dtype
    fm3 = fm_d.rearrange("b c h w -> h (b c) w")  # [rows, B, H]
    fmb = pool.tile([rows, hb, H], fdt, tag="fmb")
    nc.gpsimd.dma_start(fmb[:], fm3[:, hb:B])
    fma = pool.tile([rows, hb, H], fdt, tag="fma")
    nc.sync.dma_start(fma[:], fm3[:, 0:hb])
    ibx = None
    if not windowed:
        # ibias via the second HWDGE queue (ACT)
        ibx = pool.tile([H, 2 * TI], DT, tag="ibx")
        nc.scalar.dma_start(ibx[:], ib_d)
    return fma, fmb, ibx


def _body(nc, tc, pool, psumpool, windowed, rows, win, fm_d, ib_d, out_d,
          ident, iota_f, iotasq, psq, ones, sent, dmas=None):
    Alu = mybir.AluOpType
    if dmas is None:
        dmas = _dmas(nc, pool, windowed, rows, fm_d, ib_d)
    fma, fmb, ibx = dmas
    if not windowed:
        m2i = ibx[:, 0:TI]
        isq = ibx[:, TI:2 * TI]

    # union over batch: wide max tree, halves overlap the DMAs
    fdt = fma[:].dtype
    ma = pool.tile([rows, 2 * H], fdt, tag="ma")
    fma2 = fma[:].rearrange("p b w -> p (b w)")
    fmb2 = fmb[:].rearrange("p b w -> p (b w)")
    nc.vector.tensor_tensor(ma[:], fma2[:, 0:2 * H],
                            fma2[:, 2 * H:4 * H], op=Alu.max)
    mb = pool.tile([rows, 2 * H], fdt, tag="mb")
    nc.vector.tensor_tensor(mb[:], fmb2[:, 0:2 * H],
                            fmb2[:, 2 * H:4 * H], op=Alu.max)
    m2t = pool.tile([rows, 2 * H], fdt, tag="m2t")
    nc.vector.tensor_tensor(m2t[:], ma[:], mb[:], op=Alu.max)
    mx = pool.tile([rows, H], fdt, tag="mx")
    nc.vector.tensor_tensor(mx[:], m2t[:, 0:H], m2t[:, H:2 * H], op=Alu.max)

    # penalty: 0 where boundary, SENTINEL elsewhere. bf16 path: boundary
    # is mx >= 0.5 (truncated input); f32 path: boundary is mx > 0.5.
    pdt = fdt if windowed else DT
    pen = pool.tile([rows, H], pdt, tag="pen")
    nc.vector.tensor_scalar(out=pen[:], in0=mx[:], scalar1=0.5,
                            scalar2=sent[0:rows, 0:1],
                            op0=(Alu.is_lt if windowed else Alu.is_le),
                            op1=Alu.mult)

    # phase 1: 1D distance per row via hardware scans (state is fp32
    # internally; bf16 outputs are exact for integer distances <= 256)
    fsc = pool.tile([rows, H], pdt, tag="fsc")
    d1 = pool.tile([rows, H], pdt, tag="d1")
    if USE_SCAN:
        nc.vector.tensor_tensor_scan(fsc[:], ones[0:rows, :], pen[:],
                                     SCAN_INIT, op0=Alu.add, op1=Alu.min)
        if USE_RSCAN:
            bsc = pool.tile([rows, H], pdt, tag="bscr")
            nc.vector.tensor_tensor_scan(bsc[:], ones[0:rows, :],
                                         pen[:, ::-1], SCAN_INIT,
                                         op0=Alu.add, op1=Alu.min)
            nc.vector.tensor_tensor(d1[:], fsc[:], bsc[:, ::-1], op=Alu.min)
        else:
            v = pool.tile([rows, H], DT, tag="v")
            nc.vector.tensor_tensor(v[:], pen[:], iota_f[0:rows, :],
                                    op=Alu.add)
            vsf = _logstep_prefix_min(nc, pool, v[:], rows, +1, "s")
            bsc = pool.tile([rows, H], DT, tag="bsc")
            nc.vector.tensor_tensor(bsc[:], vsf, iota_f[0:rows, :],
                                    op=Alu.subtract)
            nc.vector.tensor_tensor(d1[:], fsc[:], bsc[:], op=Alu.min)
    else:
        u = pool.tile([rows, H], DT, tag="u")
        nc.vector.tensor_tensor(u[:], pen[:], iota_f[0:rows, :],
                                op=Alu.subtract)
        upf = _logstep_prefix_min(nc, pool, u[:], rows, -1, "p")
        nc.vector.tensor_tensor(fsc[:], upf, iota_f[0:rows, :], op=Alu.add)
        v = pool.tile([rows, H], DT, tag="v")
        nc.vector.tensor_tensor(v[:], pen[:], iota_f[0:rows, :], op=Alu.add)
        vsf = _logstep_prefix_min(nc, pool, v[:], rows, +1, "s")
        bsc = pool.tile([rows, H], DT, tag="bsc")
        nc.vector.tensor_tensor(bsc[:], vsf, iota_f[0:rows, :],
                                op=Alu.subtract)
        nc.vector.tensor_tensor(d1[:], fsc[:], bsc[:], op=Alu.min)

    # transpose d1 (PE, pass-through so PSUM dtype matches d1), square
    # it (ACT, PSUM->SBUF, converts to f32 -- exact for ints <= 256)
    pt = psumpool.tile([H, rows], pdt, tag="pt")
    nc.tensor.transpose(pt[:], d1[:], ident[:])
    t2 = pool.tile([H, rows], DT, tag="t2")  # d1[h,j]^2 at [j,h]
    nc.scalar.square(t2[:], pt[:])

    nd = 10                       # phase-2 output rows on DVE
    np_ = TI - nd                 # phase-2 output rows on Pool
    bigt = pool.tile([H, TI * win], DT, tag="bigt")
    biga = bigt[:, 0:nd * win]
    bigb = bigt[:, nd * win:TI * win]
    d2 = pool.tile([H, TI], DT, tag="d2")

    if windowed:
        # phase 2: cand[j, il, k] = d1T^2[j, il+k] + (k - WIN/2)^2; the
        # parabola row is il-independent in local coordinates, so ALL
        # output rows of an engine are one wide add over an overlapping
        # strided view of t2 (block step 1, inner step 1).
        import bass_rust
        t2ap = t2[:]

        def t2_blocks(first, count):
            return bass_rust.AP(
                t2ap.tensor, t2ap.offset + first,
                [list(t2ap.ap[0]), [1, count], [1, win]])

        nc.vector.tensor_tensor(
            biga.rearrange("p (a k) -> p a k", k=win),
            t2_blocks(0, nd),
            psq[:, 0:nd * win].rearrange("p (a k) -> p a k", k=win),
            op=Alu.add)
        nc.gpsimd.tensor_tensor(
            bigb.rearrange("p (a k) -> p a k", k=win),
            t2_blocks(nd, np_),
            psq[:, 0:np_ * win].rearrange("p (a k) -> p a k", k=win),
            op=Alu.add)
    else:
        # phase 2 via i-dependent scalars:
        # cand = (iota * -2i) + (d1T^2 + h^2); +i^2 added at the end
        t2h = pool.tile([H, rows], DT, tag="t2h")
        nc.vector.tensor_tensor(t2h[:], t2[:], iotasq[:, 0:rows], op=Alu.add)
        for il in range(nd):
            nc.vector.scalar_tensor_tensor(
                out=biga[:, il * win:(il + 1) * win], in0=iota_f[:, 0:win],
                scalar=m2i[:, il:il + 1], in1=t2h[:, 0:win],
                op0=Alu.mult, op1=Alu.add)
        for il in range(nd, TI):
            k = il - nd
            sl = slice(k * win, (k + 1) * win)
            nc.gpsimd.tensor_scalar(
                out=bigb[:, sl], in0=iota_f[:, 0:win],
                scalar1=m2i[:, il:il + 1], scalar2=None, op0=Alu.mult)
            nc.gpsimd.tensor_tensor(bigb[:, sl], bigb[:, sl],
                                    t2h[:, 0:win], op=Alu.add)

    nc.vector.tensor_reduce(
        d2[:, 0:nd], biga.rearrange("p (i h) -> p i h", h=win),
        axis=mybir.AxisListType.X, op=Alu.min)
    nc.vector.tensor_reduce(
        d2[:, nd:TI], bigb.rearrange("p (i h) -> p i h", h=win),
        axis=mybir.AxisListType.X, op=Alu.min)

    if not windowed:
        d2f = pool.tile([H, TI], DT, tag="d2f")
        nc.vector.tensor_tensor(d2f[:], d2[:], isq[:], op=Alu.add)
        d2 = d2f
    res = pool.tile([H, TI], DT, tag="res")
    nc.scalar.sqrt(res[:], d2[:])
    nc.sync.dma_start(out_d, res[:])


def _build_program(windowed: bool, repeat: int = 1, hw_loop_iters: int = 0):
    """One SPMD program. windowed=True: fm input is the per-core halo
    [B,1,HR,H] and phase 2 uses WIN-wide h-windows. windowed=False: fm is
    the full [B,1,H,H] image and phase 2 scans all 128 rows. repeat>1
    re-runs the whole body (incl. DMAs) for marginal-time measurement."""
    Alu = mybir.AluOpType
    rows = HR if windowed else H          # mask rows processed on this core
    win = WIN if windowed else H          # phase-2 candidate rows per output
    # windowed path ships the feature map as truncated bf16: the input is
    # only ever compared against 0.5 and trunc16(v) >= 0.5 <=> v > 0.5
    # (v == 0.5 exactly is host-guarded); bf16 gets the DVE 2x mode on the
    # max tree, the widest ops on the critical path.
    fdt = mybir.dt.bfloat16 if windowed else DT
    nc = bacc.Bacc("TRN2", target_bir_lowering=False, debug=False,
                   num_devices=NCORES)
    fm_d = nc.dram_tensor("fm", [B, 1, rows, H], fdt,
                          kind="ExternalInput").ap()
    ib_d = None
    if not windowed:
        # per-core side input: columns [0:TI] = -2*i, [TI:2TI] = i^2
        ib_d = nc.dram_tensor("ibias", [H, 2 * TI], DT,
                              kind="ExternalInput").ap()
    out_d = nc.dram_tensor("out", [H, TI], DT, kind="ExternalOutput").ap()

    with tile.TileContext(nc) as tc:
        with tc.tile_pool(name="main", bufs=1) as pool, \
             tc.tile_pool(name="psum", bufs=1, space="PSUM") as psumpool:

            dmas = None
            if not hw_loop_iters and repeat == 1:
                dmas = _dmas(nc, pool, windowed, rows, fm_d, ib_d)

            # constants built on device (during the first DMAs)
            cdt = mybir.dt.bfloat16 if windowed else DT
            ident = pool.tile([rows, rows], cdt, tag="ident")
            masks.make_identity(nc, ident[:])
            # sentinel via an early live Sqrt: makes the ACT func-table
            # pass load the sqrt set (which also contains Square) once,
            # instead of a mid-kernel 1.3us table switch before the final
            # sqrt. pen consumes it as a per-partition scalar.
            sent2 = pool.tile([H, 1], DT, tag="sent2")
            nc.gpsimd.memset(sent2[:], SENTINEL * SENTINEL)
            sent = pool.tile([H, 1], DT, tag="sent")
            nc.scalar.sqrt(sent[:], sent2[:])
            iota_f = iotasq = None
            if not (windowed and USE_SCAN and USE_RSCAN):
                iota_i = pool.tile([H, H], mybir.dt.int32, tag="iota_i")
                nc.gpsimd.iota(iota_i[:], pattern=[[1, H]], base=0,
                               channel_multiplier=0)
                iota_f = pool.tile([H, H], DT, tag="iota_f")
                nc.vector.tensor_copy(iota_f[:], iota_i[:])
                iotasq = pool.tile([H, H], DT, tag="iotasq")
                nc.scalar.square(iotasq[:], iota_f[:])
            if windowed:
                # psq[:, a*WIN + k] = (k - WIN/2)^2 for every block a: the
                # (i-h)^2 parabola is the same WIN-vector for every output
                # row in local coordinates, replicated TI times so phase 2
                # can consume it in one wide op per engine.
                psq_i = pool.tile([H, TI * WIN], mybir.dt.int32, tag="psq_i")
                nc.gpsimd.iota(psq_i[:], pattern=[[0, TI], [1, WIN]],
                               base=-WIN // 2, channel_multiplier=0)
                psq_f = pool.tile([H, TI * WIN], DT, tag="psq_f")
                nc.vector.tensor_copy(psq_f[:], psq_i[:])
                psq = pool.tile([H, TI * WIN], DT, tag="psq")
                nc.scalar.square(psq[:], psq_f[:])
            ones = pool.tile([rows, H], cdt, tag="ones")
            nc.gpsimd.memset(ones[:], 1.0)

            if hw_loop_iters:
                with tc.For_i(0, hw_loop_iters, 1):
                    _body(nc, tc, pool, psumpool, windowed, rows, win,
                          fm_d, ib_d, out_d, ident, iota_f, iotasq,
                          psq if windowed else None, ones, sent)
            else:
                for _rep in range(repeat):
                    _body(nc, tc, pool, psumpool, windowed, rows, win,
                          fm_d, ib_d, out_d, ident, iota_f, iotasq,
                          psq if windowed else None, ones, sent,
                          dmas=dmas if _rep == 0 else None)

    nc.compile()
    return nc


def _get_program(windowed: bool):
    key = "win" if windowed else "full"
    if key not in _CACHE:
        _CACHE[key] = _build_program(windowed)
    return _CACHE[key]


def _in_maps(feature_map: np.ndarray, windowed: bool):
    maps = []
    for c in range(NCORES):
        if windowed:
            # halo rows are true h in [16c-WIN/2, ...), zero-padded outside
            # the grid (zero rows have no boundary pixels). Shipped as
            # truncated bf16: v > 0.5 <=> trunc16(v) >= 0.5 for v != 0.5.
            lo = 16 * c - WIN // 2
            fm_c = np.zeros((B, 1, HR, H), np.float32)
            s, e = max(0, lo), min(H, lo + HR)
            fm_c[:, :, s - lo:e - lo, :] = feature_map[:, :, s:e, :]
            fm_bf = (np.ascontiguousarray(fm_c).view(np.uint32) >> 16) \
                .astype(np.uint16).view(ml_dtypes.bfloat16)
            maps.append({"fm": fm_bf})
        else:
            iv = np.arange(c * TI, (c + 1) * TI, dtype=np.float32)
            row = np.concatenate([-2.0 * iv, iv * iv])
            maps.append({
                "fm": np.ascontiguousarray(feature_map),
                "ibias": np.ascontiguousarray(
                    np.broadcast_to(row[None, :], (H, 2 * TI))),
            })
    return maps


def _run(feature_map, windowed, trace=False):
    nc = _get_program(windowed)
    out = run_bass_kernel_spmd(nc, _in_maps(feature_map, windowed),
                               list(range(NCORES)), trace=trace)
    _CACHE["last_result"] = out
    # per-core block c is [128(j), 16(i_local)] with i = 16c + i_local
    cols = np.concatenate([r["out"] for r in out.results], axis=1)
    return cols.T  # [i, j]


def kernel(feature_map: np.ndarray, _trace: bool = False):
    fm = np.ascontiguousarray(np.asarray(feature_map, dtype=np.float32))
    assert fm.shape == (B, 1, H, H), fm.shape
    if np.any(fm == np.float32(0.5)):
        # bf16-truncation trick needs v != 0.5 exactly; exact full program
        dist = _run(fm, windowed=False, trace=_trace)
        return np.ascontiguousarray(
            np.broadcast_to(dist[None, None], (B, 1, H, H))
            .astype(np.float32))
    cres = _run_cross(fm, trace=_trace).astype(np.float32)
    if np.all((cres == 0.0) | (cres == 1.0)):
        # max true distance <= 1: dist == dist^2, result is exact as-is
        return np.ascontiguousarray(
            np.broadcast_to(cres[None, None], (B, 1, H, H))
            .astype(np.float32))
    dist = _run(fm, windowed=True, trace=_trace)
    if not np.all(dist <= DMAX + 0.01):  # margin for ACT sqrt rounding
        # windowed result not provably exact -> exact full-width program
        dist = _run(fm, windowed=False, trace=_trace)
    return np.ascontiguousarray(
        np.broadcast_to(dist[None, None], (B, 1, H, H)).astype(np.float32))


# revision 17
# speedup vs baseline: 3.2937x; 3.1108x over previous
"""Distance transform kernel for Trainium2 (8 NeuronCores, SPMD).

Computes, for each pixel (i,j) of a 128x128 grid, the min Euclidean distance
to any "boundary" pixel (feature_map > 0.5, pooled over batch/channel), and
broadcasts the result over the batch dimension.

Three device programs, fastest first, each exact on a host-checkable domain:

1. "cross" (primary): binary distance transform. dist(i,j) = 0 if (i,j) is
   boundary, 1 if any 4-neighbor is boundary, else a sentinel. Exact whenever
   the true distance field is everywhere <= 1 (then dist == dist^2, so no
   sqrt is needed and the device's {0,1} output IS the final answer). The
   host validates all outputs are in {0,1}; any sentinel (true dist > 1)
   falls back to program 2. Six DVE ops total, no PE/ACT usage:
     mx  = max over batch (one innermost-axis reduce; host packs the input
           as [h, w, b] so batch is innermost)
     pen = (mx < 0.5) * SENT           (0 on boundary, SENT elsewhere)
     u   = min(pen[.,j-1], pen[.,j+1])  (free-dim shifted views, SENT-padded)
     vv  = min(pen[h-1,.], pen[h+1,.])  (partition-shifted views of the halo)
     nb  = min(u, vv)                   (best 4-neighbor)
     out = min(nb + 1, pen)             ({0, 1, ~SENT})
   Sharding: core c owns output rows [16c, 16c+16) and receives an 18-row
   halo [16c-1, 16c+17) (zero-padded outside the grid; zero rows are
   non-boundary and never win).

2. "windowed": exact separable two-phase Euclidean DT with an 8-wide phase-2
   window; exact iff max distance <= 3 (host-checked). See _build_program.

3. "full": full-width phase 2, exact for any input.

Inputs ship as truncated bf16: v > 0.5  <=>  trunc16(v) >= 0.5 for v != 0.5
(v == 0.5 exactly is host-guarded straight to program 3).

Output is batch-replicated, so no collectives are needed.
"""

import ml_dtypes
import numpy as np

import concourse.bacc as bacc
import concourse.masks as masks
import concourse.mybir as mybir
import concourse.tile as tile
from concourse.bass_utils import run_bass_kernel_spmd

H = 128          # grid height == width
B = 8            # batch
NCORES = 8
TI = H // NCORES  # output rows per core
HR = 24          # halo rows per core (windowed program)
WIN = 8          # phase-2 h-window per output row
DMAX = 3.0       # windowed result exact iff max distance <= DMAX
CR = TI + 2      # halo rows per core (cross program)

DT = mybir.dt.float32
BF = mybir.dt.bfloat16
SENTINEL = 1.0e4   # penalty for non-boundary pixels (>> max real distance)
CSENT = 16384.0    # cross-program sentinel (exact power of two in bf16)
SCAN_INIT = 1.0e9  # initial scan state
RED_INIT = 1.0e30  # pad value for log-step min fallback

import os as _os
USE_SCAN = _os.environ.get("K_USE_SCAN", "1") == "1"   # tensor_tensor_scan
USE_RSCAN = _os.environ.get("K_USE_RSCAN", "1") == "1"  # reversed-AP scan

_CACHE: dict = {}


# --------------------------------------------------------------------------
# Program 1: cross (binary DT, exact iff max dist <= 1)
# --------------------------------------------------------------------------

XC = CR + 2 * TI  # free-dim columns per (w, b): 18 center + 16 left + 16 right


def _cross_body(nc, pool, fm_d, out_d, res, idx_s, sem_out):
    Alu = mybir.AluOpType
    fmt = pool.tile([128, XC * B], BF, tag="xfmt")
    nc.sync.dma_start(fmt[:], fm_d)

    # union over batch: one reduce over the innermost (batch) axis.
    # Partition = w (all 128 lanes); free cols = 18 center halo rows,
    # then 16 left-shifted (w-1) and 16 right-shifted (w+1) output rows.
    mx = pool.tile([128, XC], BF, tag="xmx")
    nc.vector.tensor_reduce(
        mx[:], fmt[:].rearrange("p (x b) -> p x b", b=B),
        axis=mybir.AxisListType.X, op=Alu.max)

    # penalty: 0 on boundary (mx >= 0.5 on truncated bf16), SENT elsewhere
    pc = pool.tile([128, CR], BF, tag="xpc")
    nc.vector.tensor_scalar(out=pc[:], in0=mx[:, 0:CR],
                            scalar1=0.5, scalar2=CSENT,
                            op0=Alu.is_lt, op1=Alu.mult)
    plr = pool.tile([128, 2 * TI], BF, tag="xplr")
    nc.vector.tensor_scalar(out=plr[:], in0=mx[:, CR:XC],
                            scalar1=0.5, scalar2=CSENT,
                            op0=Alu.is_lt, op1=Alu.mult)

    # best 4-neighbor: vertical from shifted center views, horizontal from
    # the host-shifted copies
    vu = pool.tile([128, TI], BF, tag="xvu")
    nc.vector.tensor_tensor(vu[:], pc[:, 0:TI], pc[:, 2:CR], op=Alu.min)
    hm = pool.tile([128, TI], BF, tag="xhm")
    nc.vector.tensor_tensor(hm[:], plr[:, 0:TI], plr[:, TI:2 * TI],
                            op=Alu.min)
    nb = pool.tile([128, TI], BF, tag="xnb")
    nc.vector.tensor_tensor(nb[:], vu[:], hm[:], op=Alu.min)
    nc.vector.scalar_tensor_tensor(out=res[:], in0=nb[:], scalar=1.0,
                                   in1=pc[:, 1:TI + 1],
                                   op0=Alu.add, op1=Alu.min)
    # Output scatter (row w of the padded DRAM block <- res partition w),
    # descriptor-prepped on the otherwise-idle Pool engine: the res read is
    # deferred to the trigger, so the ~1us descriptor gen overlaps the
    # input DMA / compute, and the post-compute tail is just trigger ->
    # transfer -> completion sem (no HWDGE queue, no DGE start delay). The
    # scatter ADDs into the pre-zeroed output buffer, i.e. a plain write.
    nc.gpsimd.dma_scatter_add(
        out_d[:, 0:TI], res[:].rearrange("p (c e) -> p c e", c=1),
        idx_s[:], 128, 128, TI, elem_step=4 * TI,
        prepare_only=True, sem=sem_out, queue_num=0)
    nc.gpsimd.trigger_dma(count=None, queue_num=0)


def _build_cross(hw_loop_iters: int = 0):
    """Cross program, partition dim = w. fm input is [W, XC, B] bf16 per
    core: for each column w, the 18-row center halo plus the 16 output
    rows of columns w-1 and w+1 (host-shifted copies; zeros outside the
    grid), batch innermost. Output is [W, TI] f32 (transposed: host takes
    .T): values in {0, 1} when the result is valid, >= CSENT - 1 wherever
    the true distance exceeds 1. The output lands via a SWDGE kv_writeback
    (plain indexed write) whose descriptors are pre-generated on the
    otherwise-idle Pool engine while the input DMA is in flight, so the
    post-compute tail is just trigger -> transfer -> completion sem."""
    from concourse.tile_scheduler import PROC_NAME_TO_IDX
    nc = bacc.Bacc("TRN2", target_bir_lowering=False, debug=False,
                   num_devices=NCORES)
    fm_d = nc.dram_tensor("fm", [128, XC * B], BF, kind="ExternalInput").ap()
    # rows padded to a 256 B stride (scatter_add constraint); cols TI..63
    # are never written and ignored by the host
    out_d = nc.dram_tensor("out", [128, 64], DT, kind="ExternalOutput").ap()

    with tile.TileContext(nc) as tc:
        with tc.tile_pool(name="main", bufs=1) as pool:
            assert tc.sems is not None
            # the DMA-completion sem baked into the scatter descriptors
            # must be the DMASW lane sem that tile's pass 1 assigns
            # (consumers wait on it); this program has exactly one Pool DMA
            # inst, so the round-robin deterministically lands on lane 0
            sem_out = tc.sems[PROC_NAME_TO_IDX["DMASW0"]]
            res = pool.tile([128, TI], DT, tag="xres")
            # scatter idx k -> partition k%16, column k//16; identity map.
            # The tile spans 128 partitions (ucode contract); only the
            # first 16 are read, the rest just need in-range values.
            idx_s = pool.tile([128, 8], mybir.dt.int16, tag="xidxs")
            nc.gpsimd.memset(idx_s[:], 0)
            nc.gpsimd.iota(idx_s[0:16, :], pattern=[[16, 8]], base=0,
                           channel_multiplier=1,
                           allow_small_or_imprecise_dtypes=True)

            if hw_loop_iters:
                with tc.For_i(0, hw_loop_iters, 1):
                    _cross_body(nc, pool, fm_d, out_d, res, idx_s, sem_out)
            else:
                _cross_body(nc, pool, fm_d, out_d, res, idx_s, sem_out)

    nc.compile()
    return nc


def _cross_in_maps(feature_map: np.ndarray):
    # [B,1,H,W] f32 -> per-core [W, XC, B] truncated bf16: center halo rows
    # 16c-1 .. 16c+17, then the owned rows of columns w-1 and w+1.
    fm_bf = (np.ascontiguousarray(feature_map[:, 0]).view(np.uint32) >> 16) \
        .astype(np.uint16).view(ml_dtypes.bfloat16)  # [B, H, W]
    maps = []
    for c in range(NCORES):
        lo = TI * c - 1
        a = np.zeros((B, XC, H), ml_dtypes.bfloat16)  # [b, x, w]
        s, e = max(0, lo), min(H, lo + CR)
        a[:, s - lo:e - lo, :] = fm_bf[:, s:e, :]
        own = fm_bf[:, TI * c:TI * (c + 1), :]        # [B, TI, W]
        a[:, CR:CR + TI, 1:] = own[:, :, :-1]         # left neighbor w-1
        a[:, CR + TI:XC, :-1] = own[:, :, 1:]         # right neighbor w+1
        # -> [w, x, b] flat
        maps.append({"fm": np.ascontiguousarray(
            a.transpose(2, 1, 0)).reshape(128, XC * B)})
    return maps


def _run_cross(feature_map, trace=False):
    if "cross" not in _CACHE:
        _CACHE["cross"] = _build_cross()
    nc = _CACHE["cross"]
    out = run_bass_kernel_spmd(nc, _cross_in_maps(feature_map),
                               list(range(NCORES)), trace=trace)
    _CACHE["last_result"] = out
    # per-core block c is [w, i_local] with i = 16c + i_local
    return np.concatenate(
        [r["out"].reshape(128, 64)[:, 0:TI].T for r in out.results], axis=0)


# --------------------------------------------------------------------------
# Programs 2 & 3: windowed / full separable two-phase Euclidean DT
# --------------------------------------------------------------------------

def _logstep_prefix_min(nc, pool, src, rows, sign, tag):
    """Suffix (sign=+1) / prefix (sign=-1) min along the free dim via
    log-step shifted mins on a padded ping-pong buffer."""
    Alu = mybir.AluOpType
    pad = 64
    a = pool.tile([rows, H + pad], DT, tag=f"lsa{tag}")
    b = pool.tile([rows, H + pad], DT, tag=f"lsb{tag}")
    if sign > 0:
        data, padsl = slice(0, H), slice(H, H + pad)
    else:
        data, padsl = slice(pad, H + pad), slice(0, pad)
    nc.gpsimd.memset(a[:, padsl], RED_INIT)
    nc.gpsimd.memset(b[:, padsl], RED_INIT)
    nc.vector.tensor_copy(a[:, data], src)
    s = 1
    off = pad if sign < 0 else 0
    while s < H:
        sh = slice(off + sign * s, off + sign * s + H)
        nc.vector.tensor_tensor(b[:, data], a[:, data], a[:, sh], op=Alu.min)
        a, b = b, a
        s *= 2
    return a[:, data]


def _dmas(nc, pool, windowed, rows, fm_d, ib_d):
    """Issue the input DMAs. Emitted before the on-device constants so the
    Pool SWDGE descriptor generation isn't queued behind them."""
    hb = B // 2
    fdt = fm_d.dtype
    fm3 = fm_d.rearrange("b c h w -> h (b c) w")  # [rows, B, H]
    fmb = pool.tile([rows, hb, H], fdt, tag="fmb")
    nc.gpsimd.dma_start(fmb[:], fm3[:, hb:B])
    fma = pool.tile([rows, hb, H], fdt, tag="fma")
    nc.sync.dma_start(fma[:], fm3[:, 0:hb])
    ibx = None
    if not windowed:
        # ibias via the second HWDGE queue (ACT)
        ibx = pool.tile([H, 2 * TI], DT, tag="ibx")
        nc.scalar.dma_start(ibx[:], ib_d)
    return fma, fmb, ibx


def _body(nc, tc, pool, psumpool, windowed, rows, win, fm_d, ib_d, out_d,
          ident, iota_f, iotasq, psq, ones, sent, dmas=None):
    Alu = mybir.AluOpType
    if dmas is None:
        dmas = _dmas(nc, pool, windowed, rows, fm_d, ib_d)
    fma, fmb, ibx = dmas
    if not windowed:
        m2i = ibx[:, 0:TI]
        isq = ibx[:, TI:2 * TI]

    # union over batch: wide max tree, halves overlap the DMAs
    fdt = fma[:].dtype
    ma = pool.tile([rows, 2 * H], fdt, tag="ma")
    fma2 = fma[:].rearrange("p b w -> p (b w)")
    fmb2 = fmb[:].rearrange("p b w -> p (b w)")
    nc.vector.tensor_tensor(ma[:], fma2[:, 0:2 * H],
                            fma2[:, 2 * H:4 * H], op=Alu.max)
    mb = pool.tile([rows, 2 * H], fdt, tag="mb")
    nc.vector.tensor_tensor(mb[:], fmb2[:, 0:2 * H],
                            fmb2[:, 2 * H:4 * H], op=Alu.max)
    m2t = pool.tile([rows, 2 * H], fdt, tag="m2t")
    nc.vector.tensor_tensor(m2t[:], ma[:], mb[:], op=Alu.max)
    mx = pool.tile([rows, H], fdt, tag="mx")
    nc.vector.tensor_tensor(mx[:], m2t[:, 0:H], m2t[:, H:2 * H], op=Alu.max)

    # penalty: 0 where boundary, SENTINEL elsewhere. bf16 path: boundary
    # is mx >= 0.5 (truncated input); f32 path: boundary is mx > 0.5.
    pdt = fdt if windowed else DT
    pen = pool.tile([rows, H], pdt, tag="pen")
    nc.vector.tensor_scalar(out=pen[:], in0=mx[:], scalar1=0.5,
                            scalar2=sent[0:rows, 0:1],
                            op0=(Alu.is_lt if windowed else Alu.is_le),
                            op1=Alu.mult)

    # phase 1: 1D distance per row via hardware scans (state is fp32
    # internally; bf16 outputs are exact for integer distances <= 256)
    fsc = pool.tile([rows, H], pdt, tag="fsc")
    d1 = pool.tile([rows, H], pdt, tag="d1")
    if USE_SCAN:
        nc.vector.tensor_tensor_scan(fsc[:], ones[0:rows, :], pen[:],
                                     SCAN_INIT, op0=Alu.add, op1=Alu.min)
        if USE_RSCAN:
            bsc = pool.tile([rows, H], pdt, tag="bscr")
            nc.vector.tensor_tensor_scan(bsc[:], ones[0:rows, :],
                                         pen[:, ::-1], SCAN_INIT,
                                         op0=Alu.add, op1=Alu.min)
            nc.vector.tensor_tensor(d1[:], fsc[:], bsc[:, ::-1], op=Alu.min)
        else:
            v = pool.tile([rows, H], DT, tag="v")
            nc.vector.tensor_tensor(v[:], pen[:], iota_f[0:rows, :],
                                    op=Alu.add)
            vsf = _logstep_prefix_min(nc, pool, v[:], rows, +1, "s")
            bsc = pool.tile([rows, H], DT, tag="bsc")
            nc.vector.tensor_tensor(bsc[:], vsf, iota_f[0:rows, :],
                                    op=Alu.subtract)
            nc.vector.tensor_tensor(d1[:], fsc[:], bsc[:], op=Alu.min)
    else:
        u = pool.tile([rows, H], DT, tag="u")
        nc.vector.tensor_tensor(u[:], pen[:], iota_f[0:rows, :],
                                op=Alu.subtract)
        upf = _logstep_prefix_min(nc, pool, u[:], rows, -1, "p")
        nc.vector.tensor_tensor(fsc[:], upf, iota_f[0:rows, :], op=Alu.add)
        v = pool.tile([rows, H], DT, tag="v")
        nc.vector.tensor_tensor(v[:], pen[:], iota_f[0:rows, :], op=Alu.add)
        vsf = _logstep_prefix_min(nc, pool, v[:], rows, +1, "s")
        bsc = pool.tile([rows, H], DT, tag="bsc")
        nc.vector.tensor_tensor(bsc[:], vsf, iota_f[0:rows, :],
                                op=Alu.subtract)
        nc.vector.tensor_tensor(d1[:], fsc[:], bsc[:], op=Alu.min)

    # transpose d1 (PE, pass-through so PSUM dtype matches d1), square
    # it (ACT, PSUM->SBUF, converts to f32 -- exact for ints <= 256)
    pt = psumpool.tile([H, rows], pdt, tag="pt")
    nc.tensor.transpose(pt[:], d1[:], ident[:])
    t2 = pool.tile([H, rows], DT, tag="t2")  # d1[h,j]^2 at [j,h]
    nc.scalar.square(t2[:], pt[:])

    nd = 10                       # phase-2 output rows on DVE
    np_ = TI - nd                 # phase-2 output rows on Pool
    bigt = pool.tile([H, TI * win], DT, tag="bigt")
    biga = bigt[:, 0:nd * win]
    bigb = bigt[:, nd * win:TI * win]
    d2 = pool.tile([H, TI], DT, tag="d2")

    if windowed:
        # phase 2: cand[j, il, k] = d1T^2[j, il+k] + (k - WIN/2)^2; the
        # parabola row is il-independent in local coordinates, so ALL
        # output rows of an engine are one wide add over an overlapping
        # strided view of t2 (block step 1, inner step 1).
        import bass_rust
        t2ap = t2[:]

        def t2_blocks(first, count):
            return bass_rust.AP(
                t2ap.tensor, t2ap.offset + first,
                [list(t2ap.ap[0]), [1, count], [1, win]])

        nc.vector.tensor_tensor(
            biga.rearrange("p (a k) -> p a k", k=win),
            t2_blocks(0, nd),
            psq[:, 0:nd * win].rearrange("p (a k) -> p a k", k=win),
            op=Alu.add)
        nc.gpsimd.tensor_tensor(
            bigb.rearrange("p (a k) -> p a k", k=win),
            t2_blocks(nd, np_),
            psq[:, 0:np_ * win].rearrange("p (a k) -> p a k", k=win),
            op=Alu.add)
    else:
        # phase 2 via i-dependent scalars:
        # cand = (iota * -2i) + (d1T^2 + h^2); +i^2 added at the end
        t2h = pool.tile([H, rows], DT, tag="t2h")
        nc.vector.tensor_tensor(t2h[:], t2[:], iotasq[:, 0:rows], op=Alu.add)
        for il in range(nd):
            nc.vector.scalar_tensor_tensor(
                out=biga[:, il * win:(il + 1) * win], in0=iota_f[:, 0:win],
                scalar=m2i[:, il:il + 1], in1=t2h[:, 0:win],
                op0=Alu.mult, op1=Alu.add)
        for il in range(nd, TI):
            k = il - nd
            sl = slice(k * win, (k + 1) * win)
            nc.gpsimd.tensor_scalar(
                out=bigb[:, sl], in0=iota_f[:, 0:win],
                scalar1=m2i[:, il:il + 1], scalar2=None, op0=Alu.mult)
            nc.gpsimd.tensor_tensor(bigb[:, sl], bigb[:, sl],
                                    t2h[:, 0:win], op=Alu.add)

    nc.vector.tensor_reduce(
        d2[:, 0:nd], biga.rearrange("p (i h) -> p i h", h=win),
        axis=mybir.AxisListType.X, op=Alu.min)
    nc.vector.tensor_reduce(
        d2[:, nd:TI], bigb.rearrange("p (i h) -> p i h", h=win),
        axis=mybir.AxisListType.X, op=Alu.min)

    if not windowed:
        d2f = pool.tile([H, TI], DT, tag="d2f")
        nc.vector.tensor_tensor(d2f[:], d2[:], isq[:], op=Alu.add)
        d2 = d2f
    res = pool.tile([H, TI], DT, tag="res")
    nc.scalar.sqrt(res[:], d2[:])
    nc.sync.dma_start(out_d, res[:])


def _build_program(windowed: bool, repeat: int = 1, hw_loop_iters: int = 0):
    """One SPMD program. windowed=True: fm input is the per-core halo
    [B,1,HR,H] and phase 2 uses WIN-wide h-windows. windowed=False: fm is
    the full [B,1,H,H] image and phase 2 scans all 128 rows. repeat>1
    re-runs the whole body (incl. DMAs) for marginal-time measurement."""
    Alu = mybir.AluOpType
    rows = HR if windowed else H          # mask rows processed on this core
    win = WIN if windowed else H          # phase-2 candidate rows per output
    # windowed path ships the feature map as truncated bf16: the input is
    # only ever compared against 0.5 and trunc16(v) >= 0.5 <=> v > 0.5
    # (v == 0.5 exactly is host-guarded); bf16 gets the DVE 2x mode on the
    # max tree, the widest ops on the critical path.
    fdt = mybir.dt.bfloat16 if windowed else DT
    nc = bacc.Bacc("TRN2", target_bir_lowering=False, debug=False,
                   num_devices=NCORES)
    fm_d = nc.dram_tensor("fm", [B, 1, rows, H], fdt,
                          kind="ExternalInput").ap()
    ib_d = None
    if not windowed:
        # per-core side input: columns [0:TI] = -2*i, [TI:2TI] = i^2
        ib_d = nc.dram_tensor("ibias", [H, 2 * TI], DT,
                              kind="ExternalInput").ap()
    out_d = nc.dram_tensor("out", [H, TI], DT, kind="ExternalOutput").ap()

    with tile.TileContext(nc) as tc:
        with tc.tile_pool(name="main", bufs=1) as pool, \
             tc.tile_pool(name="psum", bufs=1, space="PSUM") as psumpool:

            dmas = None
            if not hw_loop_iters and repeat == 1:
                dmas = _dmas(nc, pool, windowed, rows, fm_d, ib_d)

            # constants built on device (during the first DMAs)
            cdt = mybir.dt.bfloat16 if windowed else DT
            ident = pool.tile([rows, rows], cdt, tag="ident")
            masks.make_identity(nc, ident[:])
            # sentinel via an early live Sqrt: makes the ACT func-table
            # pass load the sqrt set (which also contains Square) once,
            # instead of a mid-kernel 1.3us table switch before the final
            # sqrt. pen consumes it as a per-partition scalar.
            sent2 = pool.tile([H, 1], DT, tag="sent2")
            nc.gpsimd.memset(sent2[:], SENTINEL * SENTINEL)
            sent = pool.tile([H, 1], DT, tag="sent")
            nc.scalar.sqrt(sent[:], sent2[:])
            iota_f = iotasq = None
            if not (windowed and USE_SCAN and USE_RSCAN):
                iota_i = pool.tile([H, H], mybir.dt.int32, tag="iota_i")
                nc.gpsimd.iota(iota_i[:], pattern=[[1, H]], base=0,
                               channel_multiplier=0)
                iota_f = pool.tile([H, H], DT, tag="iota_f")
                nc.vector.tensor_copy(iota_f[:], iota_i[:])
                iotasq = pool.tile([H, H], DT, tag="iotasq")
                nc.scalar.square(iotasq[:], iota_f[:])
            if windowed:
                # psq[:, a*WIN + k] = (k - WIN/2)^2 for every block a: the
                # (i-h)^2 parabola is the same WIN-vector for every output
                # row in local coordinates, replicated TI times so phase 2
                # can consume it in one wide op per engine.
                psq_i = pool.tile([H, TI * WIN], mybir.dt.int32, tag="psq_i")
                nc.gpsimd.iota(psq_i[:], pattern=[[0, TI], [1, WIN]],
                               base=-WIN // 2, channel_multiplier=0)
                psq_f = pool.tile([H, TI * WIN], DT, tag="psq_f")
                nc.vector.tensor_copy(psq_f[:], psq_i[:])
                psq = pool.tile([H, TI * WIN], DT, tag="psq")
                nc.scalar.square(psq[:], psq_f[:])
            ones = pool.tile([rows, H], cdt, tag="ones")
            nc.gpsimd.memset(ones[:], 1.0)

            if hw_loop_iters:
                with tc.For_i(0, hw_loop_iters, 1):
                    _body(nc, tc, pool, psumpool, windowed, rows, win,
                          fm_d, ib_d, out_d, ident, iota_f, iotasq,
                          psq if windowed else None, ones, sent)
            else:
                for _rep in range(repeat):
                    _body(nc, tc, pool, psumpool, windowed, rows, win,
                          fm_d, ib_d, out_d, ident, iota_f, iotasq,
                          psq if windowed else None, ones, sent,
                          dmas=dmas if _rep == 0 else None)

    nc.compile()
    return nc


def _get_program(windowed: bool):
    key = "win" if windowed else "full"
    if key not in _CACHE:
        _CACHE[key] = _build_program(windowed)
    return _CACHE[key]


def _in_maps(feature_map: np.ndarray, windowed: bool):
    maps = []
    for c in range(NCORES):
        if windowed:
            # halo rows are true h in [16c-WIN/2, ...), zero-padded outside
            # the grid (zero rows have no boundary pixels). Shipped as
            # truncated bf16: v > 0.5 <=> trunc16(v) >= 0.5 for v != 0.5.
            lo = 16 * c - WIN // 2
            fm_c = np.zeros((B, 1, HR, H), np.float32)
            s, e = max(0, lo), min(H, lo + HR)
            fm_c[:, :, s - lo:e - lo, :] = feature_map[:, :, s:e, :]
            fm_bf = (np.ascontiguousarray(fm_c).view(np.uint32) >> 16) \
                .astype(np.uint16).view(ml_dtypes.bfloat16)
            maps.append({"fm": fm_bf})
        else:
            iv = np.arange(c * TI, (c + 1) * TI, dtype=np.float32)
            row = np.concatenate([-2.0 * iv, iv * iv])
            maps.append({
                "fm": np.ascontiguousarray(feature_map),
                "ibias": np.ascontiguousarray(
                    np.broadcast_to(row[None, :], (H, 2 * TI))),
            })
    return maps


def _run(feature_map, windowed, trace=False):
    nc = _get_program(windowed)
    out = run_bass_kernel_spmd(nc, _in_maps(feature_map, windowed),
                               list(range(NCORES)), trace=trace)
    _CACHE["last_result"] = out
    # per-core block c is [128(j), 16(i_local)] with i = 16c + i_local
    cols = np.concatenate([r["out"] for r in out.results], axis=1)
    return cols.T  # [i, j]


def kernel(feature_map: np.ndarray, _trace: bool = False):
    fm = np.ascontiguousarray(np.asarray(feature_map, dtype=np.float32))
    assert fm.shape == (B, 1, H, H), fm.shape
    if np.any(fm == np.float32(0.5)):
        # bf16-truncation trick needs v != 0.5 exactly; exact full program
        dist = _run(fm, windowed=False, trace=_trace)
        return np.ascontiguousarray(
            np.broadcast_to(dist[None, None], (B, 1, H, H))
            .astype(np.float32))
    cres = _run_cross(fm, trace=_trace).astype(np.float32)
    if np.all((cres == 0.0) | (cres == 1.0)):
        # max true distance <= 1: dist == dist^2, result is exact as-is
        return np.ascontiguousarray(
            np.broadcast_to(cres[None, None], (B, 1, H, H))
            .astype(np.float32))
    dist = _run(fm, windowed=True, trace=_trace)
    if not np.all(dist <= DMAX + 0.01):  # margin for ACT sqrt rounding
        # windowed result not provably exact -> exact full-width program
        dist = _run(fm, windowed=False, trace=_trace)
    return np.ascontiguousarray(
        np.broadcast_to(dist[None, None], (B, 1, H, H)).astype(np.float32))
